# revision 4
# baseline (speedup 1.0000x reference)
"""Circular pooling Trainium2 Bass kernel.

Computes the equivalent of the reference "circular pool":
  input  x: (64, 64, 15, 15, 200) f32  (B, C, H, W, Band)
  output  : (64, 64, 13, 13, 100) f32

The op is fully separable:
  band: pairs (2k, 2k+1) averaged                         (200 -> 100)
  W   : pairs (w, w+1) for w in 0..5 and 8..13 averaged,
        center col 7 passed through                       (15 -> 13)
  H   : same structure as W                               (15 -> 13)

Kernel strategy per core (pure batch parallelism, 8 examples/core):
  tile = (one example, 8 channels): SBUF partitions = (c, h) = 8*15 = 120
    free dim = (w, k) = 15*200 = 3000 contiguous in DRAM
  - DVE tensor_add: band pairs (strided reads)  (120, 3000) -> (120, 1500)
  - DVE tensor_add x2 + ACT copy (center col):  W pool      -> (120, 1300)
  - TensorE matmul with a block-diagonal (120, 104) weight: H pool
    (H-axis 0.5 scales folded into the weights)            -> PSUM (104, 1300)
  - ScalarE activation(Copy, scale): applies remaining band*W scale while
    evacuating PSUM -> SBUF, then DMA out.
"""

import numpy as np

B_FULL = 64
N_CORES = 8
B_CORE = B_FULL // N_CORES  # 8 examples per core
C = 64
H = 15
W = 15
K = 200
KO = K // 2  # 100 bands out
HO = 13
WO = 13
CCHUNK = 8
N_CTILES = C // CCHUNK  # 8
P_IN = CCHUNK * H  # 120 partitions
P_OUT = CCHUNK * HO  # 104 partitions
FIN = W * K  # 3000
FMID = W * KO  # 1500
FOUT = WO * KO  # 1300

_CACHE = {}


def _h_weights() -> np.ndarray:
    """Block-diagonal (P_IN, P_OUT) matmul weights implementing the H pool.

    Per channel block A (15 in-rows, 13 out-rows):
      out 0..5   = 0.5*(row i + row i+1), i = 0..5
      out 6      = row 7 (protected center)
      out 7..12  = 0.5*(row 8+i + row 9+i), i = 0..5
    """
    A = np.zeros((H, HO), np.float32)
    for i in range(6):
        A[i, i] = 0.5
        A[i + 1, i] = 0.5
    A[7, 6] = 1.0
    for i in range(6):
        A[8 + i, 7 + i] = 0.5
        A[9 + i, 7 + i] = 0.5
    Wm = np.zeros((P_IN, P_OUT), np.float32)
    for c in range(CCHUNK):
        Wm[c * H : (c + 1) * H, c * HO : (c + 1) * HO] = A
    return Wm


def _build_nc():
    import concourse.bacc as bacc
    import concourse.mybir as mybir
    from concourse.tile import TileContext

    # Bacc (not raw Bass): its compile() pipeline legalizes multi-wait
    # instructions (move_matmul_waits_to_ldweights / event semaphores),
    # without which walrus codegen fails with "Too many sync wait commands".
    nc = bacc.Bacc(None, target_bir_lowering=False)
    x = nc.dram_tensor(
        "x", [B_CORE, C, H, W, K], mybir.dt.float32, kind="ExternalInput"
    )
    out = nc.dram_tensor(
        "out", [B_CORE, C, HO, WO, KO], mybir.dt.float32, kind="ExternalOutput"
    )
    wdram = nc.inline_tensor(_h_weights(), name="hweights")

    fCOPY = mybir.ActivationFunctionType.Copy

    with TileContext(nc) as tc:
        with (
            tc.tile_pool(name="const", bufs=1) as const_pool,
            tc.tile_pool(name="xin", bufs=3) as in_pool,
            tc.tile_pool(name="y", bufs=2) as y_pool,
            tc.tile_pool(name="z", bufs=2) as z_pool,
            tc.tile_pool(name="o", bufs=3) as out_pool,
            tc.tile_pool(name="ps", bufs=2, space="PSUM") as psum_pool,
        ):
            wtile = const_pool.tile([P_IN, P_OUT], mybir.dt.float32)
            nc.sync.dma_start(out=wtile, in_=wdram[:, :])

            for b in range(B_CORE):
                for ci in range(N_CTILES):
                    xt = in_pool.tile([P_IN, FIN], mybir.dt.float32)
                    src = x[b, ci * CCHUNK : (ci + 1) * CCHUNK]
                    nc.sync.dma_start(
                        out=xt, in_=src.rearrange("c h w k -> (c h) (w k)")
                    )

                    # band pool: y[p, w, k'] = x[p, w, 2k'] + x[p, w, 2k'+1]
                    y = y_pool.tile([P_IN, FMID], mybir.dt.float32)
                    x3 = xt.rearrange("p (w k two) -> p w k two", two=2, k=KO)
                    y3 = y.rearrange("p (w k) -> p w k", k=KO)
                    nc.vector.tensor_add(
                        out=y3, in0=x3[:, :, :, 0], in1=x3[:, :, :, 1]
                    )

                    # W pool into z (120, 1300) = (w_out 13, k' 100)
                    z = z_pool.tile([P_IN, FOUT], mybir.dt.float32)
                    nc.vector.tensor_add(
                        out=z[:, 0:600], in0=y[:, 0:600], in1=y[:, 100:700]
                    )
                    # center col: plain copy (on DVE so z has a single
                    # producer engine — keeps the matmul's sync-wait count
                    # within the LDWEIGHTS ISA limit)
                    nc.vector.tensor_copy(out=z[:, 600:700], in_=y[:, 700:800])
                    nc.vector.tensor_add(
                        out=z[:, 700:1300], in0=y[:, 800:1400], in1=y[:, 900:1500]
                    )

                    # H pool on TensorE: psum = wtile.T @ z, f-chunks <= 512
                    ps = psum_pool.tile([P_OUT, FOUT], mybir.dt.float32)
                    for f0 in range(0, FOUT, 512):
                        f1 = min(f0 + 512, FOUT)
                        nc.tensor.matmul(
                            ps[:, f0:f1], wtile, z[:, f0:f1], start=True, stop=True
                        )

                    # Evacuate PSUM with the residual band*W scale:
                    # paired w cols: 0.5*0.5 = 0.25 ; center col 6: 0.5
                    ot = out_pool.tile([P_OUT, FOUT], mybir.dt.float32)
                    nc.scalar.activation(ot[:, 0:600], ps[:, 0:600], fCOPY, scale=0.25)
                    nc.scalar.activation(ot[:, 600:700], ps[:, 600:700], fCOPY, scale=0.5)
                    nc.scalar.activation(ot[:, 700:1300], ps[:, 700:1300], fCOPY, scale=0.25)

                    dst = out[b, ci * CCHUNK : (ci + 1) * CCHUNK]
                    nc.sync.dma_start(
                        out=dst.rearrange("c h w k -> (c h) (w k)"), in_=ot
                    )
    # Bacc defers register allocation to compile(); run_bass_via_pjrt binds
    # the BIR directly, so finalize here (compile + freeze).
    nc.finalize()
    return nc


def get_nc():
    if "nc" not in _CACHE:
        _CACHE["nc"] = _build_nc()
    return _CACHE["nc"]


def kernel(x: np.ndarray, **_unused) -> np.ndarray:
    from concourse.bass_utils import run_bass_kernel_spmd

    x = np.ascontiguousarray(x, dtype=np.float32)
    assert x.shape == (B_FULL, C, H, W, K), x.shape

    nc = get_nc()
    in_maps = [
        {"x": x[i * B_CORE : (i + 1) * B_CORE]} for i in range(N_CORES)
    ]
    res = run_bass_kernel_spmd(nc, in_maps, core_ids=list(range(N_CORES)))
    return np.concatenate([r["out"] for r in res.results], axis=0)


# revision 7
# speedup vs baseline: 7.2382x; 7.2382x over previous
"""Circular pooling Trainium2 Bass kernel.

Computes the equivalent of the reference "circular pool":
  input  x: (64, 64, 15, 15, 200) f32  (B, C, H, W, Band)
  output  : (64, 64, 13, 13, 100) f32

The op is fully separable:
  band: pairs (2k, 2k+1) averaged                         (200 -> 100)
  W   : pairs (w, w+1) for w in 0..5 and 8..13 averaged,
        center col 7 passed through                       (15 -> 13)
  H   : same structure as W                               (15 -> 13)

Kernel strategy per core (pure batch parallelism, 8 examples/core):
  tile = (one example, 8 channels): SBUF partitions = (c, h) = 8*15 = 120
    free dim = (w, k) = 15*200 = 3000 contiguous in DRAM
  - DVE tensor_add: band pairs (strided reads)  (120, 3000) -> (120, 1500)
  - DVE tensor_add x2 + ACT copy (center col):  W pool      -> (120, 1300)
  - TensorE matmul with a block-diagonal (120, 104) weight: H pool
    (H-axis 0.5 scales folded into the weights)            -> PSUM (104, 1300)
  - ScalarE activation(Copy, scale): applies remaining band*W scale while
    evacuating PSUM -> SBUF, then DMA out.
"""

import numpy as np

B_FULL = 64
N_CORES = 8
B_CORE = B_FULL // N_CORES  # 8 examples per core
C = 64
H = 15
W = 15
K = 200
KO = K // 2  # 100 bands out
HO = 13
WO = 13
CCHUNK = 8
N_CTILES = C // CCHUNK  # 8
P_IN = CCHUNK * H  # 120 partitions
P_OUT = CCHUNK * HO  # 104 partitions
FIN = W * K  # 3000
FMID = W * KO  # 1500
FOUT = WO * KO  # 1300

_CACHE = {}


def _h_weights() -> np.ndarray:
    """Block-diagonal (P_IN, P_OUT) matmul weights implementing the H pool.

    Per channel block A (15 in-rows, 13 out-rows):
      out 0..5   = 0.5*(row i + row i+1), i = 0..5
      out 6      = row 7 (protected center)
      out 7..12  = 0.5*(row 8+i + row 9+i), i = 0..5
    """
    A = np.zeros((H, HO), np.float32)
    for i in range(6):
        A[i, i] = 0.5
        A[i + 1, i] = 0.5
    A[7, 6] = 1.0
    for i in range(6):
        A[8 + i, 7 + i] = 0.5
        A[9 + i, 7 + i] = 0.5
    Wm = np.zeros((P_IN, P_OUT), np.float32)
    for c in range(CCHUNK):
        Wm[c * H : (c + 1) * H, c * HO : (c + 1) * HO] = A
    return Wm


def _build_nc(reps: int = 1):
    import concourse.bacc as bacc
    import concourse.mybir as mybir
    from concourse.tile import TileContext

    # Bacc (not raw Bass): its compile() pipeline legalizes multi-wait
    # instructions (move_matmul_waits_to_ldweights / event semaphores),
    # without which walrus codegen fails with "Too many sync wait commands".
    nc = bacc.Bacc(None, target_bir_lowering=False)
    x = nc.dram_tensor(
        "x", [B_CORE, C, H, W, K], mybir.dt.float32, kind="ExternalInput"
    )
    out = nc.dram_tensor(
        "out", [B_CORE, C, HO, WO, KO], mybir.dt.float32, kind="ExternalOutput"
    )
    wdram = nc.inline_tensor(_h_weights(), name="hweights")

    fCOPY = mybir.ActivationFunctionType.Copy

    with TileContext(nc) as tc:
        with (
            tc.tile_pool(name="const", bufs=1) as const_pool,
            tc.tile_pool(name="xin", bufs=3) as in_pool,
            tc.tile_pool(name="y", bufs=2) as y_pool,
            tc.tile_pool(name="z", bufs=2) as z_pool,
            tc.tile_pool(name="o", bufs=3) as out_pool,
            tc.tile_pool(name="ps", bufs=2, space="PSUM") as psum_pool,
        ):
            wtile = const_pool.tile([P_IN, P_OUT], mybir.dt.float32)
            nc.sync.dma_start(out=wtile, in_=wdram[:, :])

            for _rep in range(reps):
              for b in range(B_CORE):
                for ci in range(N_CTILES):
                    xt = in_pool.tile([P_IN, FIN], mybir.dt.float32)
                    src = x[b, ci * CCHUNK : (ci + 1) * CCHUNK]
                    nc.sync.dma_start(
                        out=xt, in_=src.rearrange("c h w k -> (c h) (w k)")
                    )

                    # band pool: y[p, w, k'] = x[p, w, 2k'] + x[p, w, 2k'+1]
                    y = y_pool.tile([P_IN, FMID], mybir.dt.float32)
                    x3 = xt.rearrange("p (w k two) -> p w k two", two=2, k=KO)
                    y3 = y.rearrange("p (w k) -> p w k", k=KO)
                    nc.vector.tensor_add(
                        out=y3, in0=x3[:, :, :, 0], in1=x3[:, :, :, 1]
                    )

                    # W pool into z (120, 1300) = (w_out 13, k' 100)
                    z = z_pool.tile([P_IN, FOUT], mybir.dt.float32)
                    nc.vector.tensor_add(
                        out=z[:, 0:600], in0=y[:, 0:600], in1=y[:, 100:700]
                    )
                    # center col: plain copy (on DVE so z has a single
                    # producer engine — keeps the matmul's sync-wait count
                    # within the LDWEIGHTS ISA limit)
                    nc.vector.tensor_copy(out=z[:, 600:700], in_=y[:, 700:800])
                    nc.vector.tensor_add(
                        out=z[:, 700:1300], in0=y[:, 800:1400], in1=y[:, 900:1500]
                    )

                    # H pool on TensorE: psum = wtile.T @ z, f-chunks <= 512
                    ps = psum_pool.tile([P_OUT, FOUT], mybir.dt.float32)
                    for f0 in range(0, FOUT, 512):
                        f1 = min(f0 + 512, FOUT)
                        nc.tensor.matmul(
                            ps[:, f0:f1], wtile, z[:, f0:f1], start=True, stop=True
                        )

                    # Evacuate PSUM with the residual band*W scale:
                    # paired w cols: 0.5*0.5 = 0.25 ; center col 6: 0.5
                    ot = out_pool.tile([P_OUT, FOUT], mybir.dt.float32)
                    nc.scalar.activation(ot[:, 0:600], ps[:, 0:600], fCOPY, scale=0.25)
                    nc.scalar.activation(ot[:, 600:700], ps[:, 600:700], fCOPY, scale=0.5)
                    nc.scalar.activation(ot[:, 700:1300], ps[:, 700:1300], fCOPY, scale=0.25)

                    dst = out[b, ci * CCHUNK : (ci + 1) * CCHUNK]
                    nc.sync.dma_start(
                        out=dst.rearrange("c h w k -> (c h) (w k)"), in_=ot
                    )
    # Bacc defers register allocation to compile(); run_bass_via_pjrt binds
    # the BIR directly, so finalize here (compile + freeze).
    nc.finalize()
    return nc


def get_nc(reps: int = 1):
    key = ("nc", reps)
    if key not in _CACHE:
        _CACHE[key] = _build_nc(reps)
    return _CACHE[key]


def kernel(x: np.ndarray, **_unused) -> np.ndarray:
    from concourse.bass_utils import run_bass_kernel_spmd

    x = np.ascontiguousarray(x, dtype=np.float32)
    assert x.shape == (B_FULL, C, H, W, K), x.shape

    nc = get_nc()
    in_maps = [
        {"x": x[i * B_CORE : (i + 1) * B_CORE]} for i in range(N_CORES)
    ]
    res = run_bass_kernel_spmd(nc, in_maps, core_ids=list(range(N_CORES)))
    return np.concatenate([r["out"] for r in res.results], axis=0)
